# revision 20
# baseline (speedup 1.0000x reference)
"""GNN message passing (out = A @ x, A[src,dst] = edge_attr) on 8 TRN2 NeuronCores.

Strategy: shard by src (output segment) across 8 cores. Host assigns src nodes
to (core, block, lane) slots with a balance heuristic so the per-(bucket,block)
edge-cell counts stay close to multiples of 128 (minimizes chunk padding and
the cross-core max that a single SPMD program must cover). Per core:
  - edges are binned into (src-block [128 lanes], dst-bucket [25,000 nodes])
    cells; uniform chunk counts per cell (max over cores) make one static
    program serve all 8 cores (SPMD)
  - x rows are fetched with the SWDGE dma_gather custom op (bf16, 64B payload,
    256B stride, int16 indices local to the dst bucket), round-robined over
    4 SWDGE queues so all 4 Q7 core pairs generate descriptors in parallel
  - DVE builds a one-hot scatter matrix per 128-edge chunk (iota == src_lane)
    and scales the gathered rows by edge weight into a separate buffer
  - PE matmul (one-hot^T @ messages) accumulates each block's [128, 32]
    output tile directly in PSUM across all of the block's chunks
  - one DVE copy PSUM->SBUF and one DMA writes the core's whole output
"""

import sys

sys.path.insert(0, "/opt/trn_rl_repo")

import numpy as np
import ml_dtypes

import concourse.bacc as bacc
import concourse.bass as bass
import concourse.mybir as mybir
from concourse.library_config import mlp
from concourse import bass_utils

N_NODES = 100000
D_FEAT = 32
N_CORES = 8
SRC_PER_CORE = N_NODES // N_CORES          # 12500
BLOCK = 128                                 # src nodes per block
N_BLOCKS = (SRC_PER_CORE + BLOCK - 1) // BLOCK   # 98
N_BINS = N_CORES * N_BLOCKS                 # 784
N_BUCKETS = 4
BUCKET = N_NODES // N_BUCKETS               # 25000 (fits int16 token index)
XPAD = 128                                  # bf16 row padded to 256B stride
NB = 40                                     # chunks per gather call / batch
NQ = 4                                      # SWDGE queues (Q7 core pairs)
NBUF = 8                                    # buffer rotation depth (multiple of
                                            # NQ so batches sharing a buffer
                                            # share a queue -> ordered sem)
CAP = 4 * BLOCK                             # per-cell target (4 chunks)

LAST_RESULTS = None                         # set by kernel() for test.py


NTALL = 2   # per bucket, block positions allowed 5 chunks (rest 4)
NSPLIT = 4  # batches whose gather indices ride the first (small) idx DMA
B1 = 64     # blocks flushed in the first output piece


def _position_caps():
    """cap[q, pos]: edge capacity of cell (q, pos). Position p is 'tall'
    (5 chunks) for bucket p%4 when p//4 < NTALL, else 4 chunks."""
    cap = np.full((N_BUCKETS, N_BLOCKS), 4 * BLOCK, np.int64)
    for p in range(N_BLOCKS):
        if p // 4 < NTALL and p % 4 < N_BUCKETS:
            cap[p % 4, p] = 5 * BLOCK
    return cap


def _assign_nodes(d):
    """Assign nodes to (core, block, lane) respecting per-(bucket, position)
    edge-count caps so the SPMD chunk schedule has minimal padding.

    d: [N_NODES, N_BUCKETS] per-node out-degree split by dst bucket.
    Returns bin_of_node [N], lane_of_node [N]  (bin = core * N_BLOCKS + pos).
    """
    cap_qp = _position_caps()                       # [4, 98]
    cap = np.tile(cap_qp.T, (N_CORES, 1)).astype(np.float64)   # [784, 4]
    fill = np.zeros((N_BINS, N_BUCKETS), np.float64)
    bin_of_node = np.empty(N_NODES, np.int32)
    lane_of_node = np.empty(N_NODES, np.int32)

    order = np.argsort(-d.sum(1), kind="stable")
    BIG = 1e9
    used = np.zeros(N_BINS, np.float64)
    for k, v in enumerate(order):
        r, j = divmod(k, N_BINS)
        if j == 0:
            used[:] = 0.0
        ratio = ((fill + d[v]) / cap).max(1)
        b = int(np.argmin(ratio + used))
        bin_of_node[v] = b
        lane_of_node[v] = r
        fill[b] += d[v]
        used[b] = BIG

    # swap-repair: move overflow out of over-cap cells (lane counts preserved
    # by exchanging node pairs between bins)
    icap = cap.astype(np.int64)
    ifill = fill.astype(np.int64)
    nodes_by_bin = [[] for _ in range(N_BINS)]
    for v in range(N_NODES):
        nodes_by_bin[bin_of_node[v]].append(v)
    nodes_by_bin = [np.array(l) for l in nodes_by_bin]

    def total_over():
        return np.maximum(ifill - icap, 0)

    for _ in range(3000):
        ov = total_over()
        worst = ov.sum(1)
        b0 = int(np.argmax(worst))
        if worst[b0] == 0:
            break
        q0 = int(np.argmax(ov[b0]))
        cand0 = nodes_by_bin[b0]
        best = None
        vs = cand0[np.argsort(-d[cand0, q0])[:4]]
        slack_q0 = icap[:, q0] - ifill[:, q0]
        b1s = np.argsort(-slack_q0)[:8]
        for v_star in vs:
            for b1 in b1s:
                b1 = int(b1)
                if b1 == b0:
                    continue
                cand1 = nodes_by_bin[b1]
                u_star = cand1[np.argmin(d[cand1, q0])]
                dv, du = d[v_star], d[u_star]
                new0 = ifill[b0] - dv + du
                new1 = ifill[b1] - du + dv
                gain = (np.maximum(ifill[b0] - icap[b0], 0).sum()
                        + np.maximum(ifill[b1] - icap[b1], 0).sum()
                        - np.maximum(new0 - icap[b0], 0).sum()
                        - np.maximum(new1 - icap[b1], 0).sum())
                if gain > 0 and (best is None or gain > best[0]):
                    best = (gain, v_star, b1, u_star, new0, new1)
        if best is None:
            break
        _, v_star, b1, u_star, new0, new1 = best
        ifill[b0], ifill[b1] = new0, new1
        i0 = int(np.where(nodes_by_bin[b0] == v_star)[0][0])
        i1 = int(np.where(nodes_by_bin[b1] == u_star)[0][0])
        nodes_by_bin[b0][i0] = u_star
        nodes_by_bin[b1][i1] = v_star
        bin_of_node[v_star], bin_of_node[u_star] = b1, b0
        lane_of_node[v_star], lane_of_node[u_star] = (
            lane_of_node[u_star], lane_of_node[v_star])
    return bin_of_node, lane_of_node


def _build_host_data(edge_index, edge_attr):
    src = np.asarray(edge_index[0], dtype=np.int64)
    dst = np.asarray(edge_index[1], dtype=np.int64)
    w = np.asarray(edge_attr, dtype=np.float32)
    E = src.shape[0]

    q = dst // BUCKET
    dstl = (dst - q * BUCKET).astype(np.int64)

    d = np.bincount(src * N_BUCKETS + q,
                    minlength=N_NODES * N_BUCKETS).reshape(N_NODES, N_BUCKETS)
    bin_of_node, lane_of_node = _assign_nodes(d)

    ebin = bin_of_node[src]                      # [E]
    core = ebin // N_BLOCKS
    b = (ebin % N_BLOCKS).astype(np.int64)
    srcl = lane_of_node[src].astype(np.int64)    # 0..127 lane within block

    # per (core, q, b) cell counts -> shared uniform chunk counts K[q, b]
    cell = (core * N_BUCKETS + q) * N_BLOCKS + b
    counts = np.bincount(cell, minlength=N_CORES * N_BUCKETS * N_BLOCKS)
    counts = counts.reshape(N_CORES, N_BUCKETS, N_BLOCKS)
    K = -(-counts.max(axis=0) // BLOCK)          # [N_BUCKETS, N_BLOCKS]
    K[0] = np.maximum(K[0], 1)                   # every block writes its PSUM region

    chunk_start = np.zeros((N_BUCKETS, N_BLOCKS), dtype=np.int64)
    flat = K.reshape(-1)
    chunk_start.reshape(-1)[1:] = np.cumsum(flat)[:-1]
    C = int(flat.sum())

    # schedule metadata per chunk: bucket, block
    chunk_b = np.repeat(np.tile(np.arange(N_BLOCKS), N_BUCKETS), flat)

    # per-core slot assignment (slot = chunk*128 + lane)
    order = np.argsort(cell, kind="stable")
    cs = np.bincount(cell, minlength=N_CORES * N_BUCKETS * N_BLOCKS)
    cell_first = np.zeros_like(cs)
    cell_first[1:] = np.cumsum(cs)[:-1]
    rank = np.arange(E) - cell_first[cell[order]]
    slot_base = (chunk_start[q[order], b[order]] * BLOCK)
    slot = slot_base + rank                      # within this edge's core

    per_core = []
    dstl_o = dstl[order]
    srcl_o = srcl[order]
    w_o = w[order]
    core_o = core[order]
    for c in range(N_CORES):
        m = core_o == c
        s = slot[m]
        dl = np.zeros(C * BLOCK, dtype=np.int16)
        sl = np.zeros(C * BLOCK, dtype=np.int16)
        wv = np.zeros(C * BLOCK, dtype=np.float32)
        dl[s] = dstl_o[m].astype(np.int16)
        sl[s] = srcl_o[m].astype(np.int16)
        wv[s] = w_o[m]
        per_core.append((dl, sl, wv))

    # batches: per bucket, runs of <= NB chunks; the last 64 chunks of the
    # final bucket go in size-8 batches so the pipeline tail drains fast
    batches = []   # (q, cs_chunk, n_chunks)
    pos = 0
    for qq in range(N_BUCKETS):
        nq = int(K[qq].sum())
        done = 0
        while done < nq:
            rem = nq - done
            if qq == N_BUCKETS - 1 and rem <= 64:
                n = min(8, rem)
            else:
                n = min(NB, rem)
            batches.append((qq, pos + done, n))
            done += n
        pos += nq

    # wrapped int16 gather index arrays per core: [128, C*8]
    idx_w_cores = []
    for c in range(N_CORES):
        dl = per_core[c][0]
        cols = []
        for (qq, cs_c, n) in batches:
            flat_idx = dl[cs_c * BLOCK:(cs_c + n) * BLOCK]     # slot order == j order
            wrapped = flat_idx.reshape(-1, 16).T               # [16, ni/16]
            cols.append(np.tile(wrapped, (8, 1)))              # [128, ni/16]
        idx_w_cores.append(np.concatenate(cols, axis=1))

    sched = {
        "C": C,
        "chunk_b": chunk_b,
        "batches": batches,
        "chunk_start": chunk_start,
        "K": K,
    }
    node_pos = (bin_of_node, lane_of_node)
    return sched, per_core, idx_w_cores, node_pos


def _dma_gather_raw(gpsimd, nc, out_ap, in_ap, idxs_ap, num_idxs, elem_size,
                    stride_bytes_256, queue_num=0):
    """dma_gather with a sub-256B payload (elem_size*dtype < 256B) and an
    explicit 256B-multiple row stride. Same instruction the stock wrapper
    emits; the stock wrapper just over-asserts elem alignment."""
    _in_ap = gpsimd.lower_ap_dma(in_ap, for_custom_bir_dma=True)
    _idxs_ap = gpsimd.lower_ap(idxs_ap)
    _out_ap = gpsimd.lower_ap(out_ap)
    return gpsimd.add_instruction(
        mybir.InstDMAGatherAnt(
            name=nc.get_next_instruction_name(),
            ins=[*_in_ap, _idxs_ap, gpsimd.lower_val_access(gpsimd.to_reg(num_idxs))],
            outs=[_out_ap],
            transpose=False, num_idxs=num_idxs, elem_size=elem_size,
            stride_bytes_256=stride_bytes_256, gen_mode=0, single_packet=False,
            queue_num=queue_num, sbuf_tokens_per_rank=0, sbuf_free_dim_per_rank=0,
            sbuf_free_dim_pad_per_rank=0, sbuf_byte_offset=0,
        )
    )


def _build_program(sched):
    C = sched["C"]
    chunk_b = sched["chunk_b"]
    batches = sched["batches"]
    chunk_start = sched["chunk_start"]
    K = sched["K"]
    nbatches = len(batches)
    OUTC = N_BLOCKS * D_FEAT                 # 3136

    # batch index of each chunk, then the first psem count at which all
    # chunks of blocks [0, B1) are accumulated
    batch_of_chunk = np.empty(C, np.int64)
    for i, (qq, cs_c, n) in enumerate(batches):
        batch_of_chunk[cs_c:cs_c + n] = i
    last1 = 0
    for b in range(B1):
        for qq in range(N_BUCKETS):
            if K[qq, b] > 0:
                last1 = max(last1, batch_of_chunk[
                    chunk_start[qq, b] + K[qq, b] - 1])
    T1 = int(last1) + 1

    # idx column split point: after the first NSPLIT batches
    colA = sum(n * BLOCK // 16 for (_, _, n) in batches[:NSPLIT])

    bf16 = mybir.dt.bfloat16
    f32 = mybir.dt.float32

    nc = bacc.Bacc("TRN2", target_bir_lowering=False, debug=False,
                   num_devices=N_CORES, num_swdge_queues=NQ)
    x_d = nc.dram_tensor("x", [N_NODES, XPAD], bf16, kind="ExternalInput")
    idx_d = nc.dram_tensor("idxw", [128, C * 8], mybir.dt.int16, kind="ExternalInput")
    srcl_d = nc.dram_tensor("srcl", [128, C], bf16, kind="ExternalInput")
    w_d = nc.dram_tensor("w", [128, C], bf16, kind="ExternalInput")
    iota_d = nc.dram_tensor("iota", [128, 128], bf16, kind="ExternalInput")
    out_d = nc.dram_tensor("out", [128, OUTC], f32, kind="ExternalOutput")

    from contextlib import ExitStack
    with ExitStack() as ctx:
        block = ctx.enter_context(nc.Block())
        idx_sb = ctx.enter_context(
            nc.sbuf_tensor("idx_sb", [128, C * 8], mybir.dt.int16))
        srcl_sb = ctx.enter_context(nc.sbuf_tensor("srcl_sb", [128, C], bf16))
        w_sb = ctx.enter_context(nc.sbuf_tensor("w_sb", [128, C], bf16))
        iota_sb = ctx.enter_context(nc.sbuf_tensor("iota_sb", [128, 128], bf16))
        g_sb = ctx.enter_context(
            nc.sbuf_tensor("g_sb", [128, NBUF, NB * D_FEAT], bf16))
        g2_sb = ctx.enter_context(
            nc.sbuf_tensor("g2_sb", [128, NBUF, NB * D_FEAT], bf16))
        wm_sb = ctx.enter_context(
            nc.sbuf_tensor("wm_sb", [128, NBUF, NB * 128], bf16))
        out_sb = ctx.enter_context(nc.sbuf_tensor("out_sb", [128, OUTC], f32))
        ps = ctx.enter_context(nc.psum_tensor("ps", [128, OUTC], f32))
        ioA = ctx.enter_context(nc.semaphore("ioA"))
        ioB = ctx.enter_context(nc.semaphore("ioB"))
        io = ctx.enter_context(nc.semaphore("io"))
        gsems = [ctx.enter_context(nc.semaphore(f"gsem{i}")) for i in range(NBUF)]
        wsem = ctx.enter_context(nc.semaphore("wsem"))
        psem = ctx.enter_context(nc.semaphore("psem"))
        fin = ctx.enter_context(nc.semaphore("fin"))

        @block.sync
        def _(sync):
            sync.dma_start(idx_sb[:, :colA], idx_d[:, :colA]).then_inc(ioA, 16)
            sync.dma_start(srcl_sb[:], srcl_d[:]).then_inc(io, 16)
            sync.dma_start(w_sb[:], w_d[:]).then_inc(io, 16)
            sync.dma_start(iota_sb[:], iota_d[:]).then_inc(io, 16)
            sync.dma_start(idx_sb[:, colA:], idx_d[:, colA:]).then_inc(ioB, 16)
            sync.wait_ge(fin, 1)
            sync.dma_start(out_d[:, :B1 * D_FEAT],
                           out_sb[:, :B1 * D_FEAT]).then_inc(io, 16)
            sync.wait_ge(fin, 2)
            sync.dma_start(out_d[:, B1 * D_FEAT:],
                           out_sb[:, B1 * D_FEAT:]).then_inc(io, 16)
            sync.wait_ge(io, 80)

        @block.gpsimd
        def _(gpsimd):
            gpsimd.load_library(mlp)
            icol = 0
            for i, (qq, cs_c, n) in enumerate(batches):
                if i == 0:
                    gpsimd.wait_ge(ioA, 16)
                elif i == NSPLIT:
                    gpsimd.wait_ge(ioB, 16)
                if i >= NBUF:
                    # scale of batch i-NBUF has fully read g_sb[buf]
                    gpsimd.wait_ge(wsem, 2 * (i - NBUF) + 3)
                ni = n * BLOCK
                buf = i % NBUF
                _dma_gather_raw(
                    gpsimd, nc,
                    out_ap=g_sb[:, buf, :n * D_FEAT].rearrange(
                        "p (n e) -> p n e", e=D_FEAT),
                    in_ap=x_d[qq * BUCKET:(qq + 1) * BUCKET, :D_FEAT],
                    idxs_ap=idx_sb[:, icol:icol + ni // 16],
                    num_idxs=ni, elem_size=D_FEAT,
                    stride_bytes_256=(XPAD * 2) // 256,
                    queue_num=i % NQ,
                ).then_inc(gsems[buf], 16)
                icol += ni // 16

        @block.vector
        def _(vector):
            vector.memset(ps[:], 0.0).then_inc(wsem, 1)
            vector.wait_ge(io, 48)
            for i, (qq, cs_c, n) in enumerate(batches):
                buf = i % NBUF
                if i >= NBUF:
                    # matmuls of batch i-NBUF have read wm_sb/g2_sb[buf]
                    vector.wait_ge(psem, i - NBUF + 1)
                w3 = wm_sb[:, buf, :n * 128].rearrange("p (n s) -> p n s", s=128)
                vector.tensor_tensor(
                    out=w3,
                    in0=iota_sb[:, None, :].broadcast_to([128, n, 128]),
                    in1=srcl_sb[:, cs_c:cs_c + n, None].broadcast_to([128, n, 128]),
                    op=mybir.AluOpType.is_equal,
                ).then_inc(wsem, 1)
                vector.wait_ge(gsems[buf], 16 * (i // NBUF + 1))
                g3 = g_sb[:, buf, :n * D_FEAT].rearrange("p (n e) -> p n e", e=D_FEAT)
                g4 = g2_sb[:, buf, :n * D_FEAT].rearrange("p (n e) -> p n e", e=D_FEAT)
                vector.tensor_tensor(
                    out=g4, in0=g3,
                    in1=w_sb[:, cs_c:cs_c + n, None].broadcast_to([128, n, D_FEAT]),
                    op=mybir.AluOpType.mult,
                ).then_inc(wsem, 1)

        @block.scalar
        def _(scalar):
            scalar.wait_ge(psem, T1)
            scalar.copy(out=out_sb[:, :B1 * D_FEAT],
                        in_=ps[:, :B1 * D_FEAT]).then_inc(fin, 1)
            scalar.wait_ge(psem, nbatches)
            scalar.copy(out=out_sb[:, B1 * D_FEAT:],
                        in_=ps[:, B1 * D_FEAT:]).then_inc(fin, 1)

        @block.tensor
        def _(tensor):
            for i, (qq, cs_c, n) in enumerate(batches):
                buf = i % NBUF
                tensor.wait_ge(wsem, 2 * (i + 1) + 1)
                for k in range(n):
                    c = cs_c + k
                    off = int(chunk_b[c]) * D_FEAT
                    mm = nc.tensor.matmul(
                        out=ps[:, off:off + D_FEAT],
                        lhsT=wm_sb[:, buf, k * 128:(k + 1) * 128],
                        rhs=g2_sb[:, buf, k * D_FEAT:(k + 1) * D_FEAT],
                        start=False, stop=False,
                        skip_group_check=True,
                    )
                mm.then_inc(psem, 1)


    nc.compile()
    return nc


def kernel(edge_index, edge_attr, x):
    sched, per_core, idx_w_cores, node_pos = _build_host_data(edge_index, edge_attr)
    C = sched["C"]

    x_bf = np.zeros((N_NODES, XPAD), dtype=ml_dtypes.bfloat16)
    x_bf[:, :D_FEAT] = np.asarray(x, dtype=np.float32).astype(ml_dtypes.bfloat16)
    iota = np.tile(np.arange(128, dtype=np.float32).astype(ml_dtypes.bfloat16),
                   (128, 1))

    nc = _build_program(sched)

    in_maps = []
    for c in range(N_CORES):
        dl, sl, wv = per_core[c]
        in_maps.append({
            "x": x_bf,
            "idxw": idx_w_cores[c],
            "srcl": sl.reshape(C, BLOCK).T.astype(ml_dtypes.bfloat16).copy(),
            "w": wv.reshape(C, BLOCK).T.astype(ml_dtypes.bfloat16).copy(),
            "iota": iota,
        })

    res = bass_utils.run_bass_kernel_spmd(nc, in_maps, core_ids=list(range(N_CORES)))
    global LAST_RESULTS
    LAST_RESULTS = res

    bin_of_node, lane_of_node = node_pos
    out = np.empty((N_NODES, D_FEAT), dtype=np.float32)
    core_of_node = bin_of_node // N_BLOCKS
    block_of_node = bin_of_node % N_BLOCKS
    for c in range(N_CORES):
        o = res.results[c]["out"]                      # [128, 98*32]
        o = o.reshape(128, N_BLOCKS, D_FEAT)
        m = core_of_node == c
        out[np.where(m)[0]] = o[lane_of_node[m], block_of_node[m]]
    return out
